# revision 9
# baseline (speedup 1.0000x reference)
"""Trainium2 Bass kernel for nn_AttentionBlock (GroupNorm -> QKV 1x1 -> spatial
self-attention -> out-proj + residual), sharded over 8 NeuronCores.

Sharding: data-parallel over batch (2) x query-block (4). Each core gets its
batch image with pixel columns rolled so its 1024 queries are columns 0:1024
(attention + GroupNorm are permutation-invariant over key pixels), computes
K/V over all 4096 keys, and emits its (512, 1024) output slice.

Structure (v3):
- x shipped as fp8 (projection + stats input, 2MB) + bf16 residual slice.
- GroupNorm statistics split across engines: scalar computes ct0's sum/sumsq
  via activation accumulators while vector runs bn_stats on ct1-3 - the
  serial bn_stats chain is the prologue critical path.
- GroupNorm folded into the QKV weights on device: GN(x) = A*x + B per
  channel, so K = (wk*A) @ x + wk@B.  Weights rescaled by A on the scalar
  engine after stats; shift terms become per-channel biases via 1-column
  matmuls.  No GN-apply pass over the activations at all.
- All projections + attention matmuls in fp8 DoubleRow; out-proj in bf16.
  Softmax scale folded into the Exp activation.
- Softmax 1/r applied after the out-projection (commutes with the channel
  matmul); r accumulated inline with the score loop so its reciprocal
  overlaps the tail of the AV accumulation.
- Inputs packed into few DMAs (each dma_start costs ~0.6us on the sync
  queue); output written as one contiguous bf16 block per query chunk.
"""

import numpy as np
import ml_dtypes

import concourse.bass as bass
import concourse.bacc as bacc
import concourse.mybir as mybir
import concourse.tile as tile

F32 = mybir.dt.float32
BF16 = mybir.dt.bfloat16
FP8 = mybir.dt.float8e4
DR = mybir.MatmulPerfMode.DoubleRow
AF = mybir.ActivationFunctionType
ALU = mybir.AluOpType

P = 128
C = 512          # channels
CT = C // P      # 4 channel tiles
CTP = CT // 2    # 2 channel pair-tiles (DoubleRow)
NK = 4096        # key pixels per batch image
KT = NK // P     # 32 key tiles
NQ = 1024        # queries per core
FD = 512         # matmul free-dim chunk
NCH = NK // FD   # 8 column chunks
QC = NQ // FD    # 2 query chunks
G = 32           # groups
GS = C // G      # 16 channels per group
EPS = 1e-5
SCALE = float(C) ** -0.5
N_CORES = 8
NWARM = 48       # PE warmup matmuls spanning the x-DMA/stats prologue

# packed constant layout (f32 columns per partition)
CST_GAM = 0
CST_BET = CT
CST_BQ = 2 * CT
CST_BOE = 3 * CT
CST_INDF = 4 * CT                  # CT*G ct-major
CST_INDB = 4 * CT + CT * G         # CT*P ct-major
CST_W = 4 * CT + CT * G + CT * P


def build_bass():
    nc = bacc.Bacc("TRN2", target_bir_lowering=False, debug=False,
                   num_devices=N_CORES)

    xq_d = nc.dram_tensor("xq", (C, NK), FP8, kind="ExternalInput").ap()
    xr_d = nc.dram_tensor("xr", (P, CT, NQ), BF16, kind="ExternalInput").ap()
    wq_d = nc.dram_tensor("wqT", (P, CTP, 2, C), FP8, kind="ExternalInput").ap()
    wk_d = nc.dram_tensor("wkT", (P, CTP, 2, C), FP8, kind="ExternalInput").ap()
    wv_d = nc.dram_tensor("wvT", (P, CTP, 2, C), FP8, kind="ExternalInput").ap()
    wo_d = nc.dram_tensor("woT", (P, CT, C), BF16, kind="ExternalInput").ap()
    cst_d = nc.dram_tensor("cst", (P, CST_W), F32, kind="ExternalInput").ap()
    out_d = nc.dram_tensor("out", (QC, P, CT, FD), BF16,
                           kind="ExternalOutput").ap()

    with tile.TileContext(nc) as tc:
        with (
            tc.tile_pool(name="px", bufs=1) as px,
            tc.tile_pool(name="pw", bufs=1) as pw,
            tc.tile_pool(name="pc", bufs=1) as pcst,
            tc.tile_pool(name="pkvq", bufs=1) as pkvq,
            tc.tile_pool(name="pe", bufs=4) as pe,
            tc.tile_pool(name="psm", bufs=2) as psm,
            tc.tile_pool(name="po", bufs=2) as po,
            tc.tile_pool(name="ps_u", bufs=4, space="PSUM") as ps_u,
            tc.tile_pool(name="ps_r", bufs=1, space="PSUM") as ps_r,
            tc.tile_pool(name="ps_s", bufs=2, space="PSUM") as ps_s,
            tc.tile_pool(name="ps_m", bufs=1, space="PSUM") as ps_m,
        ):
            # ---- PE warmup: dummy matmuls keep the HAM clock-gate at 8/8.
            # Blocks are interleaved with the stats-dependent matmuls below so
            # the PE never idles long enough to drop the clock.
            ones_sb = pcst.tile([P, P], BF16, tag="ones")
            nc.vector.memset(ones_sb, 1.0)
            warm_rhs = pcst.tile([P, FD], BF16, tag="wrm")
            nc.vector.memset(warm_rhs, 0.0)
            wsink = pcst.tile([P, 1], F32, tag="wsink")
            wps = ps_r.tile([P, FD], F32, tag="r", name="warm")

            def warm(n):
                for i in range(n):
                    nc.tensor.matmul(wps, ones_sb, warm_rhs,
                                     start=(i == 0), stop=(i == n - 1))
            warm(NWARM)

            # ---- x (fp8) load + GroupNorm statistics, pipelined ----
            xq_sb = px.tile([P, CTP, 2, NK], FP8, tag="xq")
            cstats = pcst.tile([P, CT, 2], F32, tag="cstats")
            stats = pcst.tile([P, CT, NCH, 6], F32, tag="stats")
            mv = pcst.tile([P, CT, 2], F32, tag="mv")
            XDH = 2048  # DMA chunk columns so bn_stats trails the DMA
            for ct in range(CT):
                xslc = xq_sb[:, ct // 2, ct % 2, :]
                for xc in range(NK // XDH):
                    xcols = slice(xc * XDH, (xc + 1) * XDH)
                    nc.sync.dma_start(out=xslc[:, xcols],
                                      in_=xq_d[ct * P:(ct + 1) * P, xcols])
                    for s in range(xc * (XDH // FD), (xc + 1) * (XDH // FD)):
                        nc.vector.bn_stats(out=stats[:, ct, s, :],
                                           in_=xslc[:, s * FD:(s + 1) * FD])
                nc.vector.bn_aggr(out=mv[:, ct, :], in_=stats[:, ct])
                # cstats = [mean, var + mean^2] == [sum/N, sumsq/N]
                nc.scalar.activation(out=cstats[:, ct, 1:2],
                                     in_=mv[:, ct, 0:1], func=AF.Square)
                nc.vector.tensor_tensor(cstats[:, ct, 1:2], cstats[:, ct, 1:2],
                                        mv[:, ct, 1:2], ALU.add)
                nc.vector.tensor_copy(out=cstats[:, ct, 0:1],
                                      in_=mv[:, ct, 0:1])

            # ---- weight / constant loads ----
            w_sb = {}
            for nm, d in (("wk", wk_d), ("wv", wv_d), ("wq", wq_d)):
                t = pw.tile([P, CTP, 2, C], FP8, tag=nm)
                nc.sync.dma_start(out=t, in_=d)
                w_sb[nm] = t
            wo_sb = pw.tile([P, CT, C], BF16, tag="wo")
            nc.sync.dma_start(out=wo_sb, in_=wo_d)
            cst_sb = pcst.tile([P, CST_W], F32, tag="cst")
            nc.sync.dma_start(out=cst_sb, in_=cst_d)
            xr_sb = px.tile([P, CT, NQ], BF16, tag="xr")
            nc.sync.dma_start(out=xr_sb, in_=xr_d)
            gam = cst_sb[:, CST_GAM:CST_GAM + CT]
            bet = cst_sb[:, CST_BET:CST_BET + CT]
            bqp = cst_sb[:, CST_BQ:CST_BQ + CT]
            boe = cst_sb[:, CST_BOE:CST_BOE + CT]

            def indf(ct):
                o = CST_INDF + ct * G
                return cst_sb[:, o:o + G]

            def indb(ct):
                o = CST_INDB + ct * P
                return cst_sb[:, o:o + P]

            eps_sb = pcst.tile([P, 1], F32, tag="eps")
            nc.vector.memset(eps_sb, EPS)

            # group combine: [32, 2] = sum_ct indf^T @ cstats  (weights 1/GS)
            # warm blocks between the per-ct matmuls span each cstats wait.
            WBLK = (22, 22, 10, 4)
            gps = ps_m.tile([G, 2], F32, tag="m", name="gps")
            for ct in range(CT):
                nc.tensor.matmul(gps, indf(ct), cstats[:, ct, :],
                                 start=(ct == 0), stop=(ct == CT - 1))
                warm(WBLK[ct])
            nc.vector.tensor_copy(out=wsink, in_=wps[:, 0:1])
            gsb = pcst.tile([P, 2], F32, tag="gsb")
            nc.vector.tensor_copy(out=gsb[0:G, :], in_=gps)
            # grhs = [mu_g, rstd_g], zero-padded to 128 partitions
            grhs = pcst.tile([P, 2], F32, tag="grhs")
            nc.vector.memset(grhs, 0.0)
            sq = pcst.tile([P, 1], F32, tag="sq")
            nc.scalar.activation(out=sq[0:G], in_=gsb[0:G, 0:1], func=AF.Square)
            nc.vector.tensor_tensor(sq[0:G], gsb[0:G, 1:2], sq[0:G], ALU.subtract)
            nc.scalar.activation(out=sq[0:G], in_=sq[0:G], func=AF.Sqrt,
                                 bias=eps_sb[0:G])
            nc.vector.tensor_copy(out=grhs[0:G, 0:1], in_=gsb[0:G, 0:1])
            nc.vector.reciprocal(out=grhs[0:G, 1:2], in_=sq[0:G])

            # per-channel GN scale/shift (batched): h = x*A + B
            abps = ps_m.tile([P, CT, 2], F32, tag="m", name="ab")
            for ct in range(CT):
                nc.tensor.matmul(abps[:, ct, :], indb(ct), grhs,
                                 start=True, stop=True)
            A_sb = pcst.tile([P, CT], F32, tag="A")
            B_sb = pcst.tile([P, CT], F32, tag="B")
            nc.vector.tensor_tensor(A_sb, abps[:, :, 1], gam, ALU.mult)
            nc.vector.tensor_tensor(B_sb, abps[:, :, 0], A_sb, ALU.mult)
            nc.vector.tensor_tensor(B_sb, bet, B_sb, ALU.subtract)

            # ---- fold GN into weights: w_eff = w * A (split scalar/vector) --
            weff = {}
            for nm in ("wk", "wv", "wq"):
                t = pw.tile([P, CTP, 2, C], FP8, tag=nm + "e")
                for ct in range(CT):
                    dst = t[:, ct // 2, ct % 2, :]
                    srcw = w_sb[nm][:, ct // 2, ct % 2, :]
                    on_scalar = (ct % 2 == 0) if nm == "wk" else (nm == "wv")
                    if on_scalar:
                        nc.scalar.activation(out=dst, in_=srcw, func=AF.Copy,
                                             scale=A_sb[:, ct:ct + 1])
                    else:
                        nc.vector.tensor_scalar(out=dst, in0=srcw,
                                                scalar1=A_sb[:, ct:ct + 1],
                                                scalar2=None, op0=ALU.mult)
                weff[nm] = t

            # shift consts: c_w[o] = sum_c w[o,c] * B[c]  (1-column matmuls)
            B_pr = pcst.tile([P, CT, 1], FP8, tag="Bpr")
            nc.vector.tensor_copy(out=B_pr[:, :, 0], in_=B_sb)
            kc_sb = pcst.tile([P, CT], F32, tag="kc")
            vc_sb = pcst.tile([P, CT], F32, tag="vc")
            qc_sb = pcst.tile([P, CT], F32, tag="qc")
            cps = ps_m.tile([P, 3, CT], F32, tag="m", name="consts")
            for wi, nm in enumerate(("wk", "wv", "wq")):
                for ot in range(CT):
                    for ctp in range(CTP):
                        nc.tensor.matmul(cps[:, wi, ot:ot + 1],
                                         w_sb[nm][:, ctp, :, ot * P:(ot + 1) * P],
                                         B_pr[:, 2 * ctp:2 * ctp + 2, :],
                                         perf_mode=DR,
                                         start=(ctp == 0), stop=(ctp == CTP - 1))
            nc.vector.tensor_copy(out=kc_sb, in_=cps[:, 0, :])
            nc.vector.tensor_copy(out=vc_sb, in_=cps[:, 1, :])
            nc.vector.tensor_tensor(qc_sb, cps[:, 2, :], bqp, ALU.add)
            vcb_sb = pcst.tile([P, CT], BF16, tag="vcb")
            nc.vector.tensor_copy(out=vcb_sb, in_=vc_sb)

            # ---- Q/K/Vt projections straight from fp8 x, per 512-col chunk --
            k_sb = pkvq.tile([P, CTP, 2, NK], FP8, tag="K")
            vt_sb = pkvq.tile([P, KT // 2, 2, FD], FP8, tag="Vt")
            q_sb = pkvq.tile([P, CTP, 2, NQ], FP8, tag="Q")
            onesp_sb = pcst.tile([P, 2, P], FP8, tag="onesp")
            nc.vector.memset(onesp_sb, 1.0)
            for ch in range(NCH):
                cols = slice(ch * FD, (ch + 1) * FD)
                # K chunk: K[ot, cols] = sum_ctp wk_eff[ctp][:,ot]^T @ x[ctp, cols]
                for ot in range(CT):
                    kps = ps_u.tile([P, FD], F32, tag="u")
                    for ctp in range(CTP):
                        nc.tensor.matmul(kps,
                                         weff["wk"][:, ctp, :, ot * P:(ot + 1) * P],
                                         xq_sb[:, ctp, :, cols], perf_mode=DR,
                                         start=(ctp == 0), stop=(ctp == CTP - 1))
                    nc.scalar.activation(out=k_sb[:, ot // 2, ot % 2, cols],
                                         in_=kps, func=AF.Identity,
                                         bias=kc_sb[:, ot:ot + 1])
                # Vt chunk: Vt[kt] = sum_ctp x[ctp, kt]^T @ wv_eff[ctp]
                for kk in range(FD // P):
                    kt = ch * (FD // P) + kk
                    vps = ps_u.tile([P, FD], F32, tag="u")
                    for ctp in range(CTP):
                        nc.tensor.matmul(vps,
                                         xq_sb[:, ctp, :,
                                               ch * FD + kk * P:ch * FD + (kk + 1) * P],
                                         weff["wv"][:, ctp, :, :], perf_mode=DR,
                                         start=(ctp == 0), stop=(ctp == CTP - 1))
                    nc.vector.tensor_copy(out=vt_sb[:, kt // 2, kt % 2, :],
                                          in_=vps)
                # Q chunk (first 1024 columns only); scale folded into Exp
                if ch < QC:
                    for ot in range(CT):
                        qps = ps_u.tile([P, FD], F32, tag="u")
                        for ctp in range(CTP):
                            nc.tensor.matmul(qps,
                                             weff["wq"][:, ctp, :, ot * P:(ot + 1) * P],
                                             xq_sb[:, ctp, :, cols], perf_mode=DR,
                                             start=(ctp == 0), stop=(ctp == CTP - 1))
                        nc.scalar.activation(out=q_sb[:, ot // 2, ot % 2, cols],
                                             in_=qps, func=AF.Identity,
                                             bias=qc_sb[:, ot:ot + 1])

            # V shift commutes through the softmax average:
            # boe2 = boe + wo @ constV   (1-column matmuls, off critical path)
            bps = ps_m.tile([P, CT], F32, tag="m", name="boe2")
            for ot in range(CT):
                for cv in range(CT):
                    nc.tensor.matmul(bps[:, ot:ot + 1],
                                     wo_sb[:, cv, ot * P:(ot + 1) * P],
                                     vcb_sb[:, cv:cv + 1],
                                     start=(cv == 0), stop=(cv == CT - 1))
            boe2_sb = pcst.tile([P, CT], F32, tag="boe2")
            nc.vector.tensor_tensor(boe2_sb, bps, boe, ALU.add)

            # xb = x_res + boe2 (residual + output bias), off critical path
            xb_sb = px.tile([P, CT, NQ], F32, tag="xb")
            for ot in range(CT):
                nc.vector.tensor_scalar(out=xb_sb[:, ot, :], in0=xr_sb[:, ot, :],
                                        scalar1=boe2_sb[:, ot:ot + 1],
                                        scalar2=None, op0=ALU.add)

            # ---- attention: St = K^T Q per k-tile, exp, U += Vt^T E, r += 1^T E
            # U stays unnormalized; 1/r is applied after the out-projection.
            attn_sb = pkvq.tile([P, CT, NQ], BF16, tag="attn")
            rr_sb = psm.tile([P, QC, FD], F32, tag="rr")
            for qc in range(QC):
                qcols = slice(qc * FD, (qc + 1) * FD)
                u_ps = [ps_u.tile([P, FD], F32, tag="u", name=f"u{qc}_{cv}")
                        for cv in range(CT)]
                r_ps = ps_r.tile([P, FD], F32, tag="r")
                KTP = KT // 2
                pend = []

                def emit_u(ep, ktp, qc=qc, u_ps=u_ps):
                    for cv in range(CT):
                        nc.tensor.matmul(u_ps[cv],
                                         vt_sb[:, ktp, :, cv * P:(cv + 1) * P],
                                         ep, perf_mode=DR,
                                         start=(ktp == 0), stop=(ktp == KTP - 1))

                for ktp in range(KTP):
                    ep = pe.tile([P, 2, FD], FP8, tag="e", name=f"e{qc}_{ktp}")
                    for i in range(2):
                        kt = 2 * ktp + i
                        sps = ps_s.tile([P, FD], F32, tag="s", name=f"s{qc}_{kt}")
                        for ctp in range(CTP):
                            nc.tensor.matmul(sps,
                                             k_sb[:, ctp, :, kt * P:(kt + 1) * P],
                                             q_sb[:, ctp, :, qcols],
                                             perf_mode=DR,
                                             start=(ctp == 0),
                                             stop=(ctp == CTP - 1))
                        nc.scalar.activation(out=ep[:, i, :], in_=sps,
                                             func=AF.Exp, scale=SCALE)
                    # r rides inline so it closes ~2 emit groups before AV ends
                    nc.tensor.matmul(r_ps, onesp_sb, ep, perf_mode=DR,
                                     start=(ktp == 0), stop=(ktp == KTP - 1))
                    pend.append((ep, ktp))
                    if len(pend) > 2:
                        emit_u(*pend.pop(0))
                # invert r while the PE drains the last U accumulations
                r_sb = psm.tile([P, FD], F32, tag="rsb", name=f"rs{qc}")
                nc.vector.tensor_copy(out=r_sb, in_=r_ps)
                nc.vector.reciprocal_approx_fast(out=rr_sb[:, qc, :], in_=r_sb)
                for item in pend:
                    emit_u(*item)
                # qc1 is the tail: U copies on the (now idle) scalar engine,
                # and the residual term xb*r is preloaded into the proj PSUM
                # so the epilogue is a single rescale (out = (wo@U + xb*r)/r).
                last = qc == QC - 1
                for cv in range(CT):
                    if last and cv % 2 == 0:
                        nc.scalar.activation(out=attn_sb[:, cv, qcols],
                                             in_=u_ps[cv], func=AF.Identity)
                    else:
                        nc.vector.tensor_copy(out=attn_sb[:, cv, qcols],
                                              in_=u_ps[cv])

                # output projection; qc=0's overlaps qc=1's attention on PE.
                proj_pool = ps_m if qc == 0 else ps_s
                o_all = po.tile([P, CT, FD], BF16, tag="o", name=f"o{qc}")
                for ot in range(CT):
                    ops = proj_pool.tile([P, FD], F32,
                                         tag="m" if qc == 0 else "s",
                                         name=f"proj{qc}_{ot}")
                    if last:
                        nc.vector.tensor_tensor(ops, xb_sb[:, ot, qcols],
                                                r_sb, ALU.mult)
                    for cv in range(CT):
                        nc.tensor.matmul(ops,
                                         wo_sb[:, cv, ot * P:(ot + 1) * P],
                                         attn_sb[:, cv, qcols],
                                         start=(cv == 0) and not last,
                                         stop=(cv == CT - 1),
                                         skip_group_check=last)
                    if last:
                        nc.vector.tensor_tensor(o_all[:, ot, :], ops,
                                                rr_sb[:, qc, :], ALU.mult)
                    else:
                        o_mul = po.tile([P, FD], F32, tag="om",
                                        name=f"om{qc}_{ot}")
                        nc.vector.tensor_tensor(o_mul, ops, rr_sb[:, qc, :],
                                                ALU.mult)
                        nc.vector.tensor_tensor(o_all[:, ot, :], o_mul,
                                                xb_sb[:, ot, qcols], ALU.add)
                    if ot % 2 == 1:
                        nc.sync.dma_start(out=out_d[qc, :, ot - 1:ot + 1, :],
                                          in_=o_all[:, ot - 1:ot + 1, :])
    nc.compile()
    return nc


def make_core_inputs(x, gn_w, gn_b, wq, bq, wk, bk, wv, bv, wo, bo):
    """Build the 8 per-core input maps from full inputs."""
    bf16 = ml_dtypes.bfloat16
    fp8 = ml_dtypes.float8_e4m3
    f32 = np.float32
    b = x.shape[0]
    xf = np.ascontiguousarray(np.asarray(x, f32).reshape(b, C, NK))

    def wpair(w):  # (512,512) w[o,c] -> fp8 pair layout [p, ctp, i, o]
        wT = np.asarray(w, f32).T.astype(fp8)
        return np.ascontiguousarray(
            wT.reshape(CTP, 2, P, C).transpose(2, 0, 1, 3))

    wkT, wvT, wqT = wpair(wk), wpair(wv), wpair(wq)
    woT = np.ascontiguousarray(
        np.asarray(wo, f32).T.astype(bf16).reshape(CT, P, C).transpose(1, 0, 2))

    def percol(v):  # (512,) -> (128, 4): [p, ct]
        return np.ascontiguousarray(np.asarray(v, f32).reshape(CT, P).T)

    bo_eff = percol(np.asarray(bo, np.float64)
                    + np.asarray(wo, np.float64) @ np.asarray(bv, np.float64))

    indf = np.zeros((P, CT, G), f32)
    indb = np.zeros((P, CT, P), f32)
    for ct in range(CT):
        for p in range(P):
            g = (ct * P + p) // GS
            indf[p, ct, g] = 1.0 / GS
            indb[g, ct, p] = 1.0
    cst = np.zeros((P, CST_W), f32)
    cst[:, CST_GAM:CST_GAM + CT] = percol(gn_w)
    cst[:, CST_BET:CST_BET + CT] = percol(gn_b)
    cst[:, CST_BQ:CST_BQ + CT] = percol(bq)
    cst[:, CST_BOE:CST_BOE + CT] = bo_eff
    cst[:, CST_INDF:CST_INDF + CT * G] = indf.reshape(P, CT * G)
    cst[:, CST_INDB:CST_INDB + CT * P] = indb.reshape(P, CT * P)

    shared = dict(wqT=wqT, wkT=wkT, wvT=wvT, woT=woT, cst=cst)

    in_maps = []
    for core in range(N_CORES):
        bb, qb = core // 4, core % 4
        qs = qb * NQ
        xr = np.ascontiguousarray(
            np.concatenate([xf[bb][:, qs:], xf[bb][:, :qs]], axis=1))
        xres = np.ascontiguousarray(
            xr[:, :NQ].reshape(CT, P, NQ).transpose(1, 0, 2)).astype(bf16)
        in_maps.append(dict(xq=xr.astype(fp8), xr=xres, **shared))
    return in_maps


def assemble(res, b=2):
    """Rebuild the full (b, C, 64, 64) output from per-core tile-major dumps."""
    out = np.zeros((b, C, NK), np.float32)
    for core in range(N_CORES):
        bb, qb = core // 4, core % 4
        t = np.asarray(res.results[core]["out"], np.float32)  # (QC, P, CT, FD)
        for qc in range(QC):
            for ot in range(CT):
                out[bb][ot * P:(ot + 1) * P,
                        qb * NQ + qc * FD:qb * NQ + (qc + 1) * FD] = \
                    t[qc, :, ot, :]
    return out.reshape(b, C, 64, 64)


_NC_CACHE = None


def _get_nc():
    global _NC_CACHE
    if _NC_CACHE is None:
        _NC_CACHE = build_bass()
    return _NC_CACHE


def run_on_cores(in_maps, **kw):
    from concourse.bass_utils import run_bass_kernel_spmd
    nc = _get_nc()
    return run_bass_kernel_spmd(nc, in_maps, core_ids=list(range(N_CORES)), **kw)


def kernel(**inputs):
    x = np.asarray(inputs["x"])
    in_maps = make_core_inputs(**inputs)
    res = run_on_cores(in_maps)
    return assemble(res, b=x.shape[0])


# revision 10
# speedup vs baseline: 1.0077x; 1.0077x over previous
"""Trainium2 Bass kernel for nn_AttentionBlock (GroupNorm -> QKV 1x1 -> spatial
self-attention -> out-proj + residual), sharded over 8 NeuronCores.

Sharding: data-parallel over batch (2) x query-block (4). Each core gets its
batch image with pixel columns rolled so its 1024 queries are columns 0:1024
(attention + GroupNorm are permutation-invariant over key pixels), computes
K/V over all 4096 keys, and emits its (512, 1024) output slice.

Structure (v3):
- x shipped as fp8 (projection + stats input, 2MB) + bf16 residual slice.
- GroupNorm statistics split across engines: scalar computes ct0's sum/sumsq
  via activation accumulators while vector runs bn_stats on ct1-3 - the
  serial bn_stats chain is the prologue critical path.
- GroupNorm folded into the QKV weights on device: GN(x) = A*x + B per
  channel, so K = (wk*A) @ x + wk@B.  Weights rescaled by A on the scalar
  engine after stats; shift terms become per-channel biases via 1-column
  matmuls.  No GN-apply pass over the activations at all.
- All projections + attention matmuls in fp8 DoubleRow; out-proj in bf16.
  Softmax scale folded into the Exp activation.
- Softmax 1/r applied after the out-projection (commutes with the channel
  matmul); r accumulated inline with the score loop so its reciprocal
  overlaps the tail of the AV accumulation.
- Inputs packed into few DMAs (each dma_start costs ~0.6us on the sync
  queue); output written as one contiguous bf16 block per query chunk.
"""

import numpy as np
import ml_dtypes

import concourse.bass as bass
import concourse.bacc as bacc
import concourse.mybir as mybir
import concourse.tile as tile

F32 = mybir.dt.float32
BF16 = mybir.dt.bfloat16
FP8 = mybir.dt.float8e4
DR = mybir.MatmulPerfMode.DoubleRow
AF = mybir.ActivationFunctionType
ALU = mybir.AluOpType

P = 128
C = 512          # channels
CT = C // P      # 4 channel tiles
CTP = CT // 2    # 2 channel pair-tiles (DoubleRow)
NK = 4096        # key pixels per batch image
KT = NK // P     # 32 key tiles
NQ = 1024        # queries per core
FD = 512         # matmul free-dim chunk
NCH = NK // FD   # 8 column chunks
QC = NQ // FD    # 2 query chunks
G = 32           # groups
GS = C // G      # 16 channels per group
EPS = 1e-5
SCALE = float(C) ** -0.5
N_CORES = 8
NWARM = 26       # PE warmup matmuls spanning the x-DMA/stats prologue

# packed constant layout (f32 columns per partition)
CST_GAM = 0
CST_BET = CT
CST_BQ = 2 * CT
CST_BOE = 3 * CT
CST_INDF = 4 * CT                  # CT*G ct-major
CST_INDB = 4 * CT + CT * G         # CT*P ct-major
CST_W = 4 * CT + CT * G + CT * P


def build_bass():
    nc = bacc.Bacc("TRN2", target_bir_lowering=False, debug=False,
                   num_devices=N_CORES)

    xq_d = nc.dram_tensor("xq", (C, NK), FP8, kind="ExternalInput").ap()
    xr_d = nc.dram_tensor("xr", (P, CT, NQ), BF16, kind="ExternalInput").ap()
    wq_d = nc.dram_tensor("wqT", (P, CTP, 2, C), FP8, kind="ExternalInput").ap()
    wk_d = nc.dram_tensor("wkT", (P, CTP, 2, C), FP8, kind="ExternalInput").ap()
    wv_d = nc.dram_tensor("wvT", (P, CTP, 2, C), FP8, kind="ExternalInput").ap()
    wo_d = nc.dram_tensor("woT", (P, CT, C), BF16, kind="ExternalInput").ap()
    cst_d = nc.dram_tensor("cst", (P, CST_W), F32, kind="ExternalInput").ap()
    out_d = nc.dram_tensor("out", (QC, P, CT, FD), BF16,
                           kind="ExternalOutput").ap()

    with tile.TileContext(nc) as tc:
        with (
            tc.tile_pool(name="px", bufs=1) as px,
            tc.tile_pool(name="pw", bufs=1) as pw,
            tc.tile_pool(name="pc", bufs=1) as pcst,
            tc.tile_pool(name="pkvq", bufs=1) as pkvq,
            tc.tile_pool(name="pe", bufs=4) as pe,
            tc.tile_pool(name="psm", bufs=2) as psm,
            tc.tile_pool(name="po", bufs=2) as po,
            tc.tile_pool(name="ps_u", bufs=4, space="PSUM") as ps_u,
            tc.tile_pool(name="ps_r", bufs=1, space="PSUM") as ps_r,
            tc.tile_pool(name="ps_s", bufs=2, space="PSUM") as ps_s,
            tc.tile_pool(name="ps_m", bufs=1, space="PSUM") as ps_m,
        ):
            # ---- PE warmup: dummy matmuls keep the HAM clock-gate at 8/8.
            # Blocks are interleaved with the stats-dependent matmuls below so
            # the PE never idles long enough to drop the clock.
            ones_sb = pcst.tile([P, P], BF16, tag="ones")
            nc.vector.memset(ones_sb, 1.0)
            warm_rhs = pcst.tile([P, FD], BF16, tag="wrm")
            nc.vector.memset(warm_rhs, 0.0)
            wsink = pcst.tile([P, 1], F32, tag="wsink")
            wps = ps_r.tile([P, FD], F32, tag="r", name="warm")

            def warm(n):
                for i in range(n):
                    nc.tensor.matmul(wps, ones_sb, warm_rhs,
                                     start=(i == 0), stop=(i == n - 1))
            warm(NWARM)

            # ---- x (fp8) load + GroupNorm statistics, pipelined ----
            xq_sb = px.tile([P, CTP, 2, NK], FP8, tag="xq")
            cstats = pcst.tile([P, CT, 2], F32, tag="cstats")
            stats = pcst.tile([P, CT, NCH, 6], F32, tag="stats")
            mv = pcst.tile([P, CT, 2], F32, tag="mv")
            XDH = 2048  # DMA chunk columns so bn_stats trails the DMA
            for ct in range(CT):
                xslc = xq_sb[:, ct // 2, ct % 2, :]
                for xc in range(NK // XDH):
                    xcols = slice(xc * XDH, (xc + 1) * XDH)
                    nc.sync.dma_start(out=xslc[:, xcols],
                                      in_=xq_d[ct * P:(ct + 1) * P, xcols])
                    for s in range(xc * (XDH // FD), (xc + 1) * (XDH // FD)):
                        nc.vector.bn_stats(out=stats[:, ct, s, :],
                                           in_=xslc[:, s * FD:(s + 1) * FD])
                nc.vector.bn_aggr(out=mv[:, ct, :], in_=stats[:, ct])
                # cstats = [mean, var + mean^2] == [sum/N, sumsq/N]
                nc.scalar.activation(out=cstats[:, ct, 1:2],
                                     in_=mv[:, ct, 0:1], func=AF.Square)
                nc.vector.tensor_tensor(cstats[:, ct, 1:2], cstats[:, ct, 1:2],
                                        mv[:, ct, 1:2], ALU.add)
                nc.vector.tensor_copy(out=cstats[:, ct, 0:1],
                                      in_=mv[:, ct, 0:1])

            # ---- weight / constant loads ----
            w_sb = {}
            for nm, d in (("wk", wk_d), ("wv", wv_d), ("wq", wq_d)):
                t = pw.tile([P, CTP, 2, C], FP8, tag=nm)
                nc.sync.dma_start(out=t, in_=d)
                w_sb[nm] = t
            wo_sb = pw.tile([P, CT, C], BF16, tag="wo")
            nc.sync.dma_start(out=wo_sb, in_=wo_d)
            cst_sb = pcst.tile([P, CST_W], F32, tag="cst")
            nc.sync.dma_start(out=cst_sb, in_=cst_d)
            xr_sb = px.tile([P, CT, NQ], BF16, tag="xr")
            nc.sync.dma_start(out=xr_sb, in_=xr_d)
            gam = cst_sb[:, CST_GAM:CST_GAM + CT]
            bet = cst_sb[:, CST_BET:CST_BET + CT]
            bqp = cst_sb[:, CST_BQ:CST_BQ + CT]
            boe = cst_sb[:, CST_BOE:CST_BOE + CT]

            def indf(ct):
                o = CST_INDF + ct * G
                return cst_sb[:, o:o + G]

            def indb(ct):
                o = CST_INDB + ct * P
                return cst_sb[:, o:o + P]

            eps_sb = pcst.tile([P, 1], F32, tag="eps")
            nc.vector.memset(eps_sb, EPS)

            # group combine: [32, 2] = sum_ct indf^T @ cstats  (weights 1/GS)
            # warm blocks between the per-ct matmuls span each cstats wait.
            WBLK = (18, 18, 18, 6)
            gps = ps_m.tile([G, 2], F32, tag="m", name="gps")
            for ct in range(CT):
                nc.tensor.matmul(gps, indf(ct), cstats[:, ct, :],
                                 start=(ct == 0), stop=(ct == CT - 1))
                warm(WBLK[ct])
            nc.vector.tensor_copy(out=wsink, in_=wps[:, 0:1])
            gsb = pcst.tile([P, 2], F32, tag="gsb")
            nc.vector.tensor_copy(out=gsb[0:G, :], in_=gps)
            # grhs = [mu_g, rstd_g], zero-padded to 128 partitions
            grhs = pcst.tile([P, 2], F32, tag="grhs")
            nc.vector.memset(grhs, 0.0)
            sq = pcst.tile([P, 1], F32, tag="sq")
            nc.scalar.activation(out=sq[0:G], in_=gsb[0:G, 0:1], func=AF.Square)
            nc.vector.tensor_tensor(sq[0:G], gsb[0:G, 1:2], sq[0:G], ALU.subtract)
            nc.scalar.activation(out=sq[0:G], in_=sq[0:G], func=AF.Sqrt,
                                 bias=eps_sb[0:G])
            nc.vector.tensor_copy(out=grhs[0:G, 0:1], in_=gsb[0:G, 0:1])
            nc.vector.reciprocal(out=grhs[0:G, 1:2], in_=sq[0:G])

            # per-channel GN scale/shift (batched): h = x*A + B
            abps = ps_m.tile([P, CT, 2], F32, tag="m", name="ab")
            for ct in range(CT):
                nc.tensor.matmul(abps[:, ct, :], indb(ct), grhs,
                                 start=True, stop=True)
            A_sb = pcst.tile([P, CT], F32, tag="A")
            B_sb = pcst.tile([P, CT], F32, tag="B")
            nc.vector.tensor_tensor(A_sb, abps[:, :, 1], gam, ALU.mult)
            nc.vector.tensor_tensor(B_sb, abps[:, :, 0], A_sb, ALU.mult)
            nc.vector.tensor_tensor(B_sb, bet, B_sb, ALU.subtract)

            # ---- fold GN into weights: w_eff = w * A (split scalar/vector) --
            weff = {}
            for nm in ("wk", "wv", "wq"):
                t = pw.tile([P, CTP, 2, C], FP8, tag=nm + "e")
                for ct in range(CT):
                    dst = t[:, ct // 2, ct % 2, :]
                    srcw = w_sb[nm][:, ct // 2, ct % 2, :]
                    on_scalar = (ct % 2 == 0) if nm == "wk" else (nm == "wv")
                    if on_scalar:
                        nc.scalar.activation(out=dst, in_=srcw, func=AF.Copy,
                                             scale=A_sb[:, ct:ct + 1])
                    else:
                        nc.vector.tensor_scalar(out=dst, in0=srcw,
                                                scalar1=A_sb[:, ct:ct + 1],
                                                scalar2=None, op0=ALU.mult)
                weff[nm] = t

            # shift consts: c_w[o] = sum_c w[o,c] * B[c]  (1-column matmuls)
            B_pr = pcst.tile([P, CT, 1], FP8, tag="Bpr")
            nc.vector.tensor_copy(out=B_pr[:, :, 0], in_=B_sb)
            kc_sb = pcst.tile([P, CT], F32, tag="kc")
            vc_sb = pcst.tile([P, CT], F32, tag="vc")
            qc_sb = pcst.tile([P, CT], F32, tag="qc")
            cps = ps_m.tile([P, 3, CT], F32, tag="m", name="consts")
            for wi, nm in enumerate(("wk", "wv", "wq")):
                for ot in range(CT):
                    for ctp in range(CTP):
                        nc.tensor.matmul(cps[:, wi, ot:ot + 1],
                                         w_sb[nm][:, ctp, :, ot * P:(ot + 1) * P],
                                         B_pr[:, 2 * ctp:2 * ctp + 2, :],
                                         perf_mode=DR,
                                         start=(ctp == 0), stop=(ctp == CTP - 1))
            nc.vector.tensor_copy(out=kc_sb, in_=cps[:, 0, :])
            nc.vector.tensor_copy(out=vc_sb, in_=cps[:, 1, :])
            nc.vector.tensor_tensor(qc_sb, cps[:, 2, :], bqp, ALU.add)
            vcb_sb = pcst.tile([P, CT], BF16, tag="vcb")
            nc.vector.tensor_copy(out=vcb_sb, in_=vc_sb)

            # ---- Q/K/Vt projections straight from fp8 x, per 512-col chunk --
            k_sb = pkvq.tile([P, CTP, 2, NK], FP8, tag="K")
            vt_sb = pkvq.tile([P, KT // 2, 2, FD], FP8, tag="Vt")
            q_sb = pkvq.tile([P, CTP, 2, NQ], FP8, tag="Q")
            onesp_sb = pcst.tile([P, 2, P], FP8, tag="onesp")
            nc.vector.memset(onesp_sb, 1.0)
            for ch in range(NCH):
                cols = slice(ch * FD, (ch + 1) * FD)
                # K chunk: K[ot, cols] = sum_ctp wk_eff[ctp][:,ot]^T @ x[ctp, cols]
                for ot in range(CT):
                    kps = ps_u.tile([P, FD], F32, tag="u")
                    for ctp in range(CTP):
                        nc.tensor.matmul(kps,
                                         weff["wk"][:, ctp, :, ot * P:(ot + 1) * P],
                                         xq_sb[:, ctp, :, cols], perf_mode=DR,
                                         start=(ctp == 0), stop=(ctp == CTP - 1))
                    nc.scalar.activation(out=k_sb[:, ot // 2, ot % 2, cols],
                                         in_=kps, func=AF.Identity,
                                         bias=kc_sb[:, ot:ot + 1])
                # Vt chunk: Vt[kt] = sum_ctp x[ctp, kt]^T @ wv_eff[ctp]
                for kk in range(FD // P):
                    kt = ch * (FD // P) + kk
                    vps = ps_u.tile([P, FD], F32, tag="u")
                    for ctp in range(CTP):
                        nc.tensor.matmul(vps,
                                         xq_sb[:, ctp, :,
                                               ch * FD + kk * P:ch * FD + (kk + 1) * P],
                                         weff["wv"][:, ctp, :, :], perf_mode=DR,
                                         start=(ctp == 0), stop=(ctp == CTP - 1))
                    nc.vector.tensor_copy(out=vt_sb[:, kt // 2, kt % 2, :],
                                          in_=vps)
                # Q chunk (first 1024 columns only); scale folded into Exp
                if ch < QC:
                    for ot in range(CT):
                        qps = ps_u.tile([P, FD], F32, tag="u")
                        for ctp in range(CTP):
                            nc.tensor.matmul(qps,
                                             weff["wq"][:, ctp, :, ot * P:(ot + 1) * P],
                                             xq_sb[:, ctp, :, cols], perf_mode=DR,
                                             start=(ctp == 0), stop=(ctp == CTP - 1))
                        nc.scalar.activation(out=q_sb[:, ot // 2, ot % 2, cols],
                                             in_=qps, func=AF.Identity,
                                             bias=qc_sb[:, ot:ot + 1])

            # V shift commutes through the softmax average:
            # boe2 = boe + wo @ constV   (1-column matmuls, off critical path)
            bps = ps_m.tile([P, CT], F32, tag="m", name="boe2")
            for ot in range(CT):
                for cv in range(CT):
                    nc.tensor.matmul(bps[:, ot:ot + 1],
                                     wo_sb[:, cv, ot * P:(ot + 1) * P],
                                     vcb_sb[:, cv:cv + 1],
                                     start=(cv == 0), stop=(cv == CT - 1))
            boe2_sb = pcst.tile([P, CT], F32, tag="boe2")
            nc.vector.tensor_tensor(boe2_sb, bps, boe, ALU.add)

            # xb = x_res + boe2 (residual + output bias), off critical path
            xb_sb = px.tile([P, CT, NQ], F32, tag="xb")
            for ot in range(CT):
                nc.vector.tensor_scalar(out=xb_sb[:, ot, :], in0=xr_sb[:, ot, :],
                                        scalar1=boe2_sb[:, ot:ot + 1],
                                        scalar2=None, op0=ALU.add)

            # ---- attention: St = K^T Q per k-tile, exp, U += Vt^T E, r += 1^T E
            # U stays unnormalized; 1/r is applied after the out-projection.
            attn_sb = pkvq.tile([P, CT, NQ], BF16, tag="attn")
            rr_sb = psm.tile([P, QC, FD], F32, tag="rr")
            for qc in range(QC):
                qcols = slice(qc * FD, (qc + 1) * FD)
                u_ps = [ps_u.tile([P, FD], F32, tag="u", name=f"u{qc}_{cv}")
                        for cv in range(CT)]
                r_ps = ps_r.tile([P, FD], F32, tag="r")
                KTP = KT // 2
                pend = []

                def emit_u(ep, ktp, qc=qc, u_ps=u_ps):
                    for cv in range(CT):
                        nc.tensor.matmul(u_ps[cv],
                                         vt_sb[:, ktp, :, cv * P:(cv + 1) * P],
                                         ep, perf_mode=DR,
                                         start=(ktp == 0), stop=(ktp == KTP - 1))

                for ktp in range(KTP):
                    ep = pe.tile([P, 2, FD], FP8, tag="e", name=f"e{qc}_{ktp}")
                    for i in range(2):
                        kt = 2 * ktp + i
                        sps = ps_s.tile([P, FD], F32, tag="s", name=f"s{qc}_{kt}")
                        for ctp in range(CTP):
                            nc.tensor.matmul(sps,
                                             k_sb[:, ctp, :, kt * P:(kt + 1) * P],
                                             q_sb[:, ctp, :, qcols],
                                             perf_mode=DR,
                                             start=(ctp == 0),
                                             stop=(ctp == CTP - 1))
                        nc.scalar.activation(out=ep[:, i, :], in_=sps,
                                             func=AF.Exp, scale=SCALE)
                    # r rides inline so it closes ~2 emit groups before AV ends
                    nc.tensor.matmul(r_ps, onesp_sb, ep, perf_mode=DR,
                                     start=(ktp == 0), stop=(ktp == KTP - 1))
                    pend.append((ep, ktp))
                    if len(pend) > 2:
                        emit_u(*pend.pop(0))
                # invert r while the PE drains the last U accumulations
                r_sb = psm.tile([P, FD], F32, tag="rsb", name=f"rs{qc}")
                nc.vector.tensor_copy(out=r_sb, in_=r_ps)
                nc.vector.reciprocal_approx_fast(out=rr_sb[:, qc, :], in_=r_sb)
                for item in pend:
                    emit_u(*item)
                # qc1 is the tail: U copies split scalar/vector, and the
                # residual term xb*r is preloaded into the proj PSUM banks
                # (before the U copies - it only needs r) so the epilogue is
                # a single rescale (out = (wo@U + xb*r)/r).
                last = qc == QC - 1
                proj_pool = ps_m if qc == 0 else ps_s
                pre = {}
                if last:
                    for ot in range(CT):
                        pre[ot] = proj_pool.tile([P, FD], F32, tag="s",
                                                 name=f"proj{qc}_{ot}")
                        nc.vector.tensor_tensor(pre[ot], xb_sb[:, ot, qcols],
                                                r_sb, ALU.mult)
                for cv in range(CT):
                    if last and cv % 2 == 0:
                        nc.scalar.activation(out=attn_sb[:, cv, qcols],
                                             in_=u_ps[cv], func=AF.Identity)
                    else:
                        nc.vector.tensor_copy(out=attn_sb[:, cv, qcols],
                                              in_=u_ps[cv])

                # output projection; qc=0's overlaps qc=1's attention on PE.
                o_all = po.tile([P, CT, FD], BF16, tag="o", name=f"o{qc}")
                for ot in range(CT):
                    ops = pre[ot] if last else proj_pool.tile(
                        [P, FD], F32, tag="m", name=f"proj{qc}_{ot}")
                    for cv in range(CT):
                        nc.tensor.matmul(ops,
                                         wo_sb[:, cv, ot * P:(ot + 1) * P],
                                         attn_sb[:, cv, qcols],
                                         start=(cv == 0) and not last,
                                         stop=(cv == CT - 1),
                                         skip_group_check=last)
                    if last:
                        nc.vector.tensor_tensor(o_all[:, ot, :], ops,
                                                rr_sb[:, qc, :], ALU.mult)
                    else:
                        o_mul = po.tile([P, FD], F32, tag="om",
                                        name=f"om{qc}_{ot}")
                        nc.vector.tensor_tensor(o_mul, ops, rr_sb[:, qc, :],
                                                ALU.mult)
                        nc.vector.tensor_tensor(o_all[:, ot, :], o_mul,
                                                xb_sb[:, ot, qcols], ALU.add)
                    if ot % 2 == 1:
                        nc.sync.dma_start(out=out_d[qc, :, ot - 1:ot + 1, :],
                                          in_=o_all[:, ot - 1:ot + 1, :])
    nc.compile()
    return nc


def make_core_inputs(x, gn_w, gn_b, wq, bq, wk, bk, wv, bv, wo, bo):
    """Build the 8 per-core input maps from full inputs."""
    bf16 = ml_dtypes.bfloat16
    fp8 = ml_dtypes.float8_e4m3
    f32 = np.float32
    b = x.shape[0]
    xf = np.ascontiguousarray(np.asarray(x, f32).reshape(b, C, NK))

    def wpair(w):  # (512,512) w[o,c] -> fp8 pair layout [p, ctp, i, o]
        wT = np.asarray(w, f32).T.astype(fp8)
        return np.ascontiguousarray(
            wT.reshape(CTP, 2, P, C).transpose(2, 0, 1, 3))

    wkT, wvT, wqT = wpair(wk), wpair(wv), wpair(wq)
    woT = np.ascontiguousarray(
        np.asarray(wo, f32).T.astype(bf16).reshape(CT, P, C).transpose(1, 0, 2))

    def percol(v):  # (512,) -> (128, 4): [p, ct]
        return np.ascontiguousarray(np.asarray(v, f32).reshape(CT, P).T)

    bo_eff = percol(np.asarray(bo, np.float64)
                    + np.asarray(wo, np.float64) @ np.asarray(bv, np.float64))

    indf = np.zeros((P, CT, G), f32)
    indb = np.zeros((P, CT, P), f32)
    for ct in range(CT):
        for p in range(P):
            g = (ct * P + p) // GS
            indf[p, ct, g] = 1.0 / GS
            indb[g, ct, p] = 1.0
    cst = np.zeros((P, CST_W), f32)
    cst[:, CST_GAM:CST_GAM + CT] = percol(gn_w)
    cst[:, CST_BET:CST_BET + CT] = percol(gn_b)
    cst[:, CST_BQ:CST_BQ + CT] = percol(bq)
    cst[:, CST_BOE:CST_BOE + CT] = bo_eff
    cst[:, CST_INDF:CST_INDF + CT * G] = indf.reshape(P, CT * G)
    cst[:, CST_INDB:CST_INDB + CT * P] = indb.reshape(P, CT * P)

    shared = dict(wqT=wqT, wkT=wkT, wvT=wvT, woT=woT, cst=cst)

    in_maps = []
    for core in range(N_CORES):
        bb, qb = core // 4, core % 4
        qs = qb * NQ
        xr = np.ascontiguousarray(
            np.concatenate([xf[bb][:, qs:], xf[bb][:, :qs]], axis=1))
        xres = np.ascontiguousarray(
            xr[:, :NQ].reshape(CT, P, NQ).transpose(1, 0, 2)).astype(bf16)
        in_maps.append(dict(xq=xr.astype(fp8), xr=xres, **shared))
    return in_maps


def assemble(res, b=2):
    """Rebuild the full (b, C, 64, 64) output from per-core tile-major dumps."""
    out = np.zeros((b, C, NK), np.float32)
    for core in range(N_CORES):
        bb, qb = core // 4, core % 4
        t = np.asarray(res.results[core]["out"], np.float32)  # (QC, P, CT, FD)
        for qc in range(QC):
            for ot in range(CT):
                out[bb][ot * P:(ot + 1) * P,
                        qb * NQ + qc * FD:qb * NQ + (qc + 1) * FD] = \
                    t[qc, :, ot, :]
    return out.reshape(b, C, 64, 64)


_NC_CACHE = None


def _get_nc():
    global _NC_CACHE
    if _NC_CACHE is None:
        _NC_CACHE = build_bass()
    return _NC_CACHE


def run_on_cores(in_maps, **kw):
    from concourse.bass_utils import run_bass_kernel_spmd
    nc = _get_nc()
    return run_bass_kernel_spmd(nc, in_maps, core_ids=list(range(N_CORES)), **kw)


def kernel(**inputs):
    x = np.asarray(inputs["x"])
    in_maps = make_core_inputs(**inputs)
    res = run_on_cores(in_maps)
    return assemble(res, b=x.shape[0])


# revision 11
# speedup vs baseline: 1.0087x; 1.0009x over previous
"""Trainium2 Bass kernel for nn_AttentionBlock (GroupNorm -> QKV 1x1 -> spatial
self-attention -> out-proj + residual), sharded over 8 NeuronCores.

Sharding: data-parallel over batch (2) x query-block (4). Each core gets its
batch image with pixel columns rolled so its 1024 queries are columns 0:1024
(attention + GroupNorm are permutation-invariant over key pixels), computes
K/V over all 4096 keys, and emits its (512, 1024) output slice.

Structure (v3):
- x shipped as fp8 (projection + stats input, 2MB) + bf16 residual slice.
- GroupNorm statistics split across engines: scalar computes ct0's sum/sumsq
  via activation accumulators while vector runs bn_stats on ct1-3 - the
  serial bn_stats chain is the prologue critical path.
- GroupNorm folded into the QKV weights on device: GN(x) = A*x + B per
  channel, so K = (wk*A) @ x + wk@B.  Weights rescaled by A on the scalar
  engine after stats; shift terms become per-channel biases via 1-column
  matmuls.  No GN-apply pass over the activations at all.
- All projections + attention matmuls in fp8 DoubleRow; out-proj in bf16.
  Softmax scale folded into the Exp activation.
- Softmax 1/r applied after the out-projection (commutes with the channel
  matmul); r accumulated inline with the score loop so its reciprocal
  overlaps the tail of the AV accumulation.
- Inputs packed into few DMAs (each dma_start costs ~0.6us on the sync
  queue); output written as one contiguous bf16 block per query chunk.
"""

import numpy as np
import ml_dtypes

import concourse.bass as bass
import concourse.bacc as bacc
import concourse.mybir as mybir
import concourse.tile as tile

F32 = mybir.dt.float32
BF16 = mybir.dt.bfloat16
FP8 = mybir.dt.float8e4
DR = mybir.MatmulPerfMode.DoubleRow
AF = mybir.ActivationFunctionType
ALU = mybir.AluOpType

P = 128
C = 512          # channels
CT = C // P      # 4 channel tiles
CTP = CT // 2    # 2 channel pair-tiles (DoubleRow)
NK = 4096        # key pixels per batch image
KT = NK // P     # 32 key tiles
NQ = 1024        # queries per core
FD = 512         # matmul free-dim chunk
NCH = NK // FD   # 8 column chunks
QC = NQ // FD    # 2 query chunks
G = 32           # groups
GS = C // G      # 16 channels per group
EPS = 1e-5
SCALE = float(C) ** -0.5
N_CORES = 8
NWARM = 26       # PE warmup matmuls spanning the x-DMA/stats prologue

# packed constant layout (f32 columns per partition)
CST_GAM = 0
CST_BET = CT
CST_BQ = 2 * CT
CST_BOE = 3 * CT
CST_INDF = 4 * CT                  # CT*G ct-major
CST_INDB = 4 * CT + CT * G         # CT*P ct-major
CST_W = 4 * CT + CT * G + CT * P


def build_bass():
    nc = bacc.Bacc("TRN2", target_bir_lowering=False, debug=False,
                   num_devices=N_CORES)

    xq_d = nc.dram_tensor("xq", (C, NK), FP8, kind="ExternalInput").ap()
    xr_d = nc.dram_tensor("xr", (P, CT, NQ), BF16, kind="ExternalInput").ap()
    wq_d = nc.dram_tensor("wqT", (P, CTP, 2, C), FP8, kind="ExternalInput").ap()
    wk_d = nc.dram_tensor("wkT", (P, CTP, 2, C), FP8, kind="ExternalInput").ap()
    wv_d = nc.dram_tensor("wvT", (P, CTP, 2, C), FP8, kind="ExternalInput").ap()
    wo_d = nc.dram_tensor("woT", (P, CT, C), BF16, kind="ExternalInput").ap()
    cst_d = nc.dram_tensor("cst", (P, CST_W), F32, kind="ExternalInput").ap()
    out_d = nc.dram_tensor("out", (QC, P, CT, FD), BF16,
                           kind="ExternalOutput").ap()

    with tile.TileContext(nc) as tc:
        with (
            tc.tile_pool(name="px", bufs=1) as px,
            tc.tile_pool(name="pw", bufs=1) as pw,
            tc.tile_pool(name="pc", bufs=1) as pcst,
            tc.tile_pool(name="pkvq", bufs=1) as pkvq,
            tc.tile_pool(name="pe", bufs=4) as pe,
            tc.tile_pool(name="psm", bufs=2) as psm,
            tc.tile_pool(name="po", bufs=2) as po,
            tc.tile_pool(name="ps_u", bufs=4, space="PSUM") as ps_u,
            tc.tile_pool(name="ps_r", bufs=1, space="PSUM") as ps_r,
            tc.tile_pool(name="ps_s", bufs=2, space="PSUM") as ps_s,
            tc.tile_pool(name="ps_m", bufs=1, space="PSUM") as ps_m,
        ):
            # ---- PE warmup: dummy matmuls keep the HAM clock-gate at 8/8.
            # Blocks are interleaved with the stats-dependent matmuls below so
            # the PE never idles long enough to drop the clock.
            ones_sb = pcst.tile([P, P], BF16, tag="ones")
            nc.vector.memset(ones_sb, 1.0)
            warm_rhs = pcst.tile([P, FD], BF16, tag="wrm")
            nc.vector.memset(warm_rhs, 0.0)
            wsink = pcst.tile([P, 1], F32, tag="wsink")
            wps = ps_r.tile([P, FD], F32, tag="r", name="warm")

            def warm(n):
                for i in range(n):
                    nc.tensor.matmul(wps, ones_sb, warm_rhs,
                                     start=(i == 0), stop=(i == n - 1))
            warm(NWARM)

            # ---- x (fp8) load + GroupNorm statistics, pipelined ----
            xq_sb = px.tile([P, CTP, 2, NK], FP8, tag="xq")
            cstats = pcst.tile([P, CT, 2], F32, tag="cstats")
            stats = pcst.tile([P, CT, NCH, 6], F32, tag="stats")
            mv = pcst.tile([P, CT, 2], F32, tag="mv")
            XDH = 2048  # DMA chunk columns so bn_stats trails the DMA
            for ct in range(CT):
                xslc = xq_sb[:, ct // 2, ct % 2, :]
                for xc in range(NK // XDH):
                    xcols = slice(xc * XDH, (xc + 1) * XDH)
                    nc.sync.dma_start(out=xslc[:, xcols],
                                      in_=xq_d[ct * P:(ct + 1) * P, xcols])
                    for s in range(xc * (XDH // FD), (xc + 1) * (XDH // FD)):
                        nc.vector.bn_stats(out=stats[:, ct, s, :],
                                           in_=xslc[:, s * FD:(s + 1) * FD])
                nc.vector.bn_aggr(out=mv[:, ct, :], in_=stats[:, ct])
                # cstats = [mean, var + mean^2] == [sum/N, sumsq/N]
                nc.scalar.activation(out=cstats[:, ct, 1:2],
                                     in_=mv[:, ct, 0:1], func=AF.Square)
                nc.vector.tensor_tensor(cstats[:, ct, 1:2], cstats[:, ct, 1:2],
                                        mv[:, ct, 1:2], ALU.add)
                nc.vector.tensor_copy(out=cstats[:, ct, 0:1],
                                      in_=mv[:, ct, 0:1])

            # ---- weight / constant loads ----
            w_sb = {}
            for nm, d in (("wk", wk_d), ("wv", wv_d), ("wq", wq_d)):
                t = pw.tile([P, CTP, 2, C], FP8, tag=nm)
                nc.sync.dma_start(out=t, in_=d)
                w_sb[nm] = t
            wo_sb = pw.tile([P, CT, C], BF16, tag="wo")
            nc.sync.dma_start(out=wo_sb, in_=wo_d)
            cst_sb = pcst.tile([P, CST_W], F32, tag="cst")
            nc.sync.dma_start(out=cst_sb, in_=cst_d)
            xr_sb = px.tile([P, CT, NQ], BF16, tag="xr")
            nc.sync.dma_start(out=xr_sb, in_=xr_d)
            gam = cst_sb[:, CST_GAM:CST_GAM + CT]
            bet = cst_sb[:, CST_BET:CST_BET + CT]
            bqp = cst_sb[:, CST_BQ:CST_BQ + CT]
            boe = cst_sb[:, CST_BOE:CST_BOE + CT]

            def indf(ct):
                o = CST_INDF + ct * G
                return cst_sb[:, o:o + G]

            def indb(ct):
                o = CST_INDB + ct * P
                return cst_sb[:, o:o + P]

            eps_sb = pcst.tile([P, 1], F32, tag="eps")
            nc.vector.memset(eps_sb, EPS)

            # group combine: [32, 2] = sum_ct indf^T @ cstats  (weights 1/GS)
            # warm blocks between the per-ct matmuls span each cstats wait.
            WBLK = (18, 18, 18, 6)
            gps = ps_m.tile([G, 2], F32, tag="m", name="gps")
            for ct in range(CT):
                nc.tensor.matmul(gps, indf(ct), cstats[:, ct, :],
                                 start=(ct == 0), stop=(ct == CT - 1))
                warm(WBLK[ct])
            nc.vector.tensor_copy(out=wsink, in_=wps[:, 0:1])
            gsb = pcst.tile([P, 2], F32, tag="gsb")
            nc.vector.tensor_copy(out=gsb[0:G, :], in_=gps)
            # grhs = [mu_g, rstd_g], zero-padded to 128 partitions
            grhs = pcst.tile([P, 2], F32, tag="grhs")
            nc.vector.memset(grhs, 0.0)
            sq = pcst.tile([P, 1], F32, tag="sq")
            nc.scalar.activation(out=sq[0:G], in_=gsb[0:G, 0:1], func=AF.Square)
            nc.vector.tensor_tensor(sq[0:G], gsb[0:G, 1:2], sq[0:G], ALU.subtract)
            nc.scalar.activation(out=sq[0:G], in_=sq[0:G], func=AF.Sqrt,
                                 bias=eps_sb[0:G])
            nc.vector.tensor_copy(out=grhs[0:G, 0:1], in_=gsb[0:G, 0:1])
            nc.vector.reciprocal(out=grhs[0:G, 1:2], in_=sq[0:G])

            # per-channel GN scale/shift (batched): h = x*A + B
            abps = ps_m.tile([P, CT, 2], F32, tag="m", name="ab")
            for ct in range(CT):
                nc.tensor.matmul(abps[:, ct, :], indb(ct), grhs,
                                 start=True, stop=True)
            A_sb = pcst.tile([P, CT], F32, tag="A")
            B_sb = pcst.tile([P, CT], F32, tag="B")
            nc.vector.tensor_tensor(A_sb, abps[:, :, 1], gam, ALU.mult)
            nc.vector.tensor_tensor(B_sb, abps[:, :, 0], A_sb, ALU.mult)
            nc.vector.tensor_tensor(B_sb, bet, B_sb, ALU.subtract)

            # ---- fold GN into weights: w_eff = w * A (split scalar/vector) --
            weff = {}
            for nm in ("wk", "wv", "wq"):
                t = pw.tile([P, CTP, 2, C], FP8, tag=nm + "e")
                for ct in range(CT):
                    dst = t[:, ct // 2, ct % 2, :]
                    srcw = w_sb[nm][:, ct // 2, ct % 2, :]
                    on_scalar = (ct % 2 == 0) if nm == "wk" else (nm == "wv")
                    if on_scalar:
                        nc.scalar.activation(out=dst, in_=srcw, func=AF.Copy,
                                             scale=A_sb[:, ct:ct + 1])
                    else:
                        nc.vector.tensor_scalar(out=dst, in0=srcw,
                                                scalar1=A_sb[:, ct:ct + 1],
                                                scalar2=None, op0=ALU.mult)
                weff[nm] = t

            # shift consts: c_w[o] = sum_c w[o,c] * B[c]  (1-column matmuls)
            B_pr = pcst.tile([P, CT, 1], FP8, tag="Bpr")
            nc.vector.tensor_copy(out=B_pr[:, :, 0], in_=B_sb)
            kc_sb = pcst.tile([P, CT], F32, tag="kc")
            vc_sb = pcst.tile([P, CT], F32, tag="vc")
            qc_sb = pcst.tile([P, CT], F32, tag="qc")
            cps = ps_m.tile([P, 3, CT], F32, tag="m", name="consts")
            for wi, nm in enumerate(("wk", "wv", "wq")):
                for ot in range(CT):
                    for ctp in range(CTP):
                        nc.tensor.matmul(cps[:, wi, ot:ot + 1],
                                         w_sb[nm][:, ctp, :, ot * P:(ot + 1) * P],
                                         B_pr[:, 2 * ctp:2 * ctp + 2, :],
                                         perf_mode=DR,
                                         start=(ctp == 0), stop=(ctp == CTP - 1))
            nc.vector.tensor_copy(out=kc_sb, in_=cps[:, 0, :])
            nc.vector.tensor_copy(out=vc_sb, in_=cps[:, 1, :])
            nc.vector.tensor_tensor(qc_sb, cps[:, 2, :], bqp, ALU.add)
            vcb_sb = pcst.tile([P, CT], BF16, tag="vcb")
            nc.vector.tensor_copy(out=vcb_sb, in_=vc_sb)

            # ---- Q/K/Vt projections straight from fp8 x, per 512-col chunk --
            k_sb = pkvq.tile([P, CTP, 2, NK], FP8, tag="K")
            vt_sb = pkvq.tile([P, KT // 2, 2, FD], FP8, tag="Vt")
            q_sb = pkvq.tile([P, CTP, 2, NQ], FP8, tag="Q")
            onesp_sb = pcst.tile([P, 2, P], FP8, tag="onesp")
            nc.vector.memset(onesp_sb, 1.0)
            for ch in range(NCH):
                cols = slice(ch * FD, (ch + 1) * FD)
                # K chunk: K[ot, cols] = sum_ctp wk_eff[ctp][:,ot]^T @ x[ctp, cols]
                for ot in range(CT):
                    kps = ps_u.tile([P, FD], F32, tag="u")
                    for ctp in range(CTP):
                        nc.tensor.matmul(kps,
                                         weff["wk"][:, ctp, :, ot * P:(ot + 1) * P],
                                         xq_sb[:, ctp, :, cols], perf_mode=DR,
                                         start=(ctp == 0), stop=(ctp == CTP - 1))
                    nc.scalar.activation(out=k_sb[:, ot // 2, ot % 2, cols],
                                         in_=kps, func=AF.Identity,
                                         bias=kc_sb[:, ot:ot + 1])
                # Vt chunk: Vt[kt] = sum_ctp x[ctp, kt]^T @ wv_eff[ctp]
                for kk in range(FD // P):
                    kt = ch * (FD // P) + kk
                    vps = ps_u.tile([P, FD], F32, tag="u")
                    for ctp in range(CTP):
                        nc.tensor.matmul(vps,
                                         xq_sb[:, ctp, :,
                                               ch * FD + kk * P:ch * FD + (kk + 1) * P],
                                         weff["wv"][:, ctp, :, :], perf_mode=DR,
                                         start=(ctp == 0), stop=(ctp == CTP - 1))
                    nc.vector.tensor_copy(out=vt_sb[:, kt // 2, kt % 2, :],
                                          in_=vps)
                # Q chunk (first 1024 columns only); scale folded into Exp
                if ch < QC:
                    for ot in range(CT):
                        qps = ps_u.tile([P, FD], F32, tag="u")
                        for ctp in range(CTP):
                            nc.tensor.matmul(qps,
                                             weff["wq"][:, ctp, :, ot * P:(ot + 1) * P],
                                             xq_sb[:, ctp, :, cols], perf_mode=DR,
                                             start=(ctp == 0), stop=(ctp == CTP - 1))
                        nc.scalar.activation(out=q_sb[:, ot // 2, ot % 2, cols],
                                             in_=qps, func=AF.Identity,
                                             bias=qc_sb[:, ot:ot + 1])

            # V shift commutes through the softmax average:
            # boe2 = boe + wo @ constV   (1-column matmuls, off critical path)
            bps = ps_m.tile([P, CT], F32, tag="m", name="boe2")
            for ot in range(CT):
                for cv in range(CT):
                    nc.tensor.matmul(bps[:, ot:ot + 1],
                                     wo_sb[:, cv, ot * P:(ot + 1) * P],
                                     vcb_sb[:, cv:cv + 1],
                                     start=(cv == 0), stop=(cv == CT - 1))
            boe2_sb = pcst.tile([P, CT], F32, tag="boe2")
            nc.vector.tensor_tensor(boe2_sb, bps, boe, ALU.add)

            # xb = x_res + boe2 (residual + output bias), off critical path
            xb_sb = px.tile([P, CT, NQ], F32, tag="xb")
            for ot in range(CT):
                nc.vector.tensor_scalar(out=xb_sb[:, ot, :], in0=xr_sb[:, ot, :],
                                        scalar1=boe2_sb[:, ot:ot + 1],
                                        scalar2=None, op0=ALU.add)

            # ---- attention: St = K^T Q per k-tile, exp, U += Vt^T E, r += 1^T E
            # U stays unnormalized; 1/r is applied after the out-projection.
            attn_sb = pkvq.tile([P, CT, NQ], BF16, tag="attn")
            rr_sb = psm.tile([P, QC, FD], F32, tag="rr")
            for qc in range(QC):
                qcols = slice(qc * FD, (qc + 1) * FD)
                u_ps = [ps_u.tile([P, FD], F32, tag="u", name=f"u{qc}_{cv}")
                        for cv in range(CT)]
                r_ps = ps_r.tile([P, FD], F32, tag="r")
                KTP = KT // 2
                pend = []

                def emit_u(ep, ktp, qc=qc, u_ps=u_ps):
                    for cv in range(CT):
                        nc.tensor.matmul(u_ps[cv],
                                         vt_sb[:, ktp, :, cv * P:(cv + 1) * P],
                                         ep, perf_mode=DR,
                                         start=(ktp == 0), stop=(ktp == KTP - 1))

                for ktp in range(KTP):
                    ep = pe.tile([P, 2, FD], FP8, tag="e", name=f"e{qc}_{ktp}")
                    for i in range(2):
                        kt = 2 * ktp + i
                        sps = ps_s.tile([P, FD], F32, tag="s", name=f"s{qc}_{kt}")
                        for ctp in range(CTP):
                            nc.tensor.matmul(sps,
                                             k_sb[:, ctp, :, kt * P:(kt + 1) * P],
                                             q_sb[:, ctp, :, qcols],
                                             perf_mode=DR,
                                             start=(ctp == 0),
                                             stop=(ctp == CTP - 1))
                        nc.scalar.activation(out=ep[:, i, :], in_=sps,
                                             func=AF.Exp, scale=SCALE)
                    # r rides inline so it closes ~2 emit groups before AV ends
                    nc.tensor.matmul(r_ps, onesp_sb, ep, perf_mode=DR,
                                     start=(ktp == 0), stop=(ktp == KTP - 1))
                    pend.append((ep, ktp))
                    if len(pend) > 2:
                        emit_u(*pend.pop(0))
                # invert r while the PE drains the last U accumulations
                r_sb = psm.tile([P, FD], F32, tag="rsb", name=f"rs{qc}")
                nc.vector.tensor_copy(out=r_sb, in_=r_ps)
                nc.vector.reciprocal_approx_fast(out=rr_sb[:, qc, :], in_=r_sb)
                for item in pend:
                    emit_u(*item)
                # qc1 is the tail: U copies split scalar/vector, and the
                # residual term xb*r is preloaded into the proj PSUM banks
                # (before the U copies - it only needs r) so the epilogue is
                # a single rescale (out = (wo@U + xb*r)/r).
                last = qc == QC - 1
                # spread proj PSUM tiles over the banks that are free here
                if last:
                    pp = [(ps_s, "s"), (ps_s, "s"), (ps_m, "m"), (ps_r, "r")]
                else:
                    pp = [(ps_m, "m"), (ps_r, "r"), (ps_m, "m"), (ps_r, "r")]
                pre = {}
                if last:
                    for ot in range(CT):
                        pool, tg = pp[ot]
                        pre[ot] = pool.tile([P, FD], F32, tag=tg,
                                            name=f"proj{qc}_{ot}")
                        nc.vector.tensor_tensor(pre[ot], xb_sb[:, ot, qcols],
                                                r_sb, ALU.mult)
                for cv in range(CT):
                    if last and cv % 2 == 0:
                        nc.scalar.activation(out=attn_sb[:, cv, qcols],
                                             in_=u_ps[cv], func=AF.Identity)
                    else:
                        nc.vector.tensor_copy(out=attn_sb[:, cv, qcols],
                                              in_=u_ps[cv])

                # output projection; qc=0's overlaps qc=1's attention on PE.
                o_all = po.tile([P, CT, FD], BF16, tag="o", name=f"o{qc}")
                for ot in range(CT):
                    ops = pre[ot] if last else pp[ot][0].tile(
                        [P, FD], F32, tag=pp[ot][1], name=f"proj{qc}_{ot}")
                    for cv in range(CT):
                        nc.tensor.matmul(ops,
                                         wo_sb[:, cv, ot * P:(ot + 1) * P],
                                         attn_sb[:, cv, qcols],
                                         start=(cv == 0) and not last,
                                         stop=(cv == CT - 1),
                                         skip_group_check=last)
                    if last:
                        nc.vector.tensor_tensor(o_all[:, ot, :], ops,
                                                rr_sb[:, qc, :], ALU.mult)
                    else:
                        o_mul = po.tile([P, FD], F32, tag="om",
                                        name=f"om{qc}_{ot}")
                        nc.vector.tensor_tensor(o_mul, ops, rr_sb[:, qc, :],
                                                ALU.mult)
                        nc.vector.tensor_tensor(o_all[:, ot, :], o_mul,
                                                xb_sb[:, ot, qcols], ALU.add)
                    if ot % 2 == 1:
                        nc.sync.dma_start(out=out_d[qc, :, ot - 1:ot + 1, :],
                                          in_=o_all[:, ot - 1:ot + 1, :])
    nc.compile()
    return nc


def make_core_inputs(x, gn_w, gn_b, wq, bq, wk, bk, wv, bv, wo, bo):
    """Build the 8 per-core input maps from full inputs."""
    bf16 = ml_dtypes.bfloat16
    fp8 = ml_dtypes.float8_e4m3
    f32 = np.float32
    b = x.shape[0]
    xf = np.ascontiguousarray(np.asarray(x, f32).reshape(b, C, NK))

    def wpair(w):  # (512,512) w[o,c] -> fp8 pair layout [p, ctp, i, o]
        wT = np.asarray(w, f32).T.astype(fp8)
        return np.ascontiguousarray(
            wT.reshape(CTP, 2, P, C).transpose(2, 0, 1, 3))

    wkT, wvT, wqT = wpair(wk), wpair(wv), wpair(wq)
    woT = np.ascontiguousarray(
        np.asarray(wo, f32).T.astype(bf16).reshape(CT, P, C).transpose(1, 0, 2))

    def percol(v):  # (512,) -> (128, 4): [p, ct]
        return np.ascontiguousarray(np.asarray(v, f32).reshape(CT, P).T)

    bo_eff = percol(np.asarray(bo, np.float64)
                    + np.asarray(wo, np.float64) @ np.asarray(bv, np.float64))

    indf = np.zeros((P, CT, G), f32)
    indb = np.zeros((P, CT, P), f32)
    for ct in range(CT):
        for p in range(P):
            g = (ct * P + p) // GS
            indf[p, ct, g] = 1.0 / GS
            indb[g, ct, p] = 1.0
    cst = np.zeros((P, CST_W), f32)
    cst[:, CST_GAM:CST_GAM + CT] = percol(gn_w)
    cst[:, CST_BET:CST_BET + CT] = percol(gn_b)
    cst[:, CST_BQ:CST_BQ + CT] = percol(bq)
    cst[:, CST_BOE:CST_BOE + CT] = bo_eff
    cst[:, CST_INDF:CST_INDF + CT * G] = indf.reshape(P, CT * G)
    cst[:, CST_INDB:CST_INDB + CT * P] = indb.reshape(P, CT * P)

    shared = dict(wqT=wqT, wkT=wkT, wvT=wvT, woT=woT, cst=cst)

    in_maps = []
    for core in range(N_CORES):
        bb, qb = core // 4, core % 4
        qs = qb * NQ
        xr = np.ascontiguousarray(
            np.concatenate([xf[bb][:, qs:], xf[bb][:, :qs]], axis=1))
        xres = np.ascontiguousarray(
            xr[:, :NQ].reshape(CT, P, NQ).transpose(1, 0, 2)).astype(bf16)
        in_maps.append(dict(xq=xr.astype(fp8), xr=xres, **shared))
    return in_maps


def assemble(res, b=2):
    """Rebuild the full (b, C, 64, 64) output from per-core tile-major dumps."""
    out = np.zeros((b, C, NK), np.float32)
    for core in range(N_CORES):
        bb, qb = core // 4, core % 4
        t = np.asarray(res.results[core]["out"], np.float32)  # (QC, P, CT, FD)
        for qc in range(QC):
            for ot in range(CT):
                out[bb][ot * P:(ot + 1) * P,
                        qb * NQ + qc * FD:qb * NQ + (qc + 1) * FD] = \
                    t[qc, :, ot, :]
    return out.reshape(b, C, 64, 64)


_NC_CACHE = None


def _get_nc():
    global _NC_CACHE
    if _NC_CACHE is None:
        _NC_CACHE = build_bass()
    return _NC_CACHE


def run_on_cores(in_maps, **kw):
    from concourse.bass_utils import run_bass_kernel_spmd
    nc = _get_nc()
    return run_bass_kernel_spmd(nc, in_maps, core_ids=list(range(N_CORES)), **kw)


def kernel(**inputs):
    x = np.asarray(inputs["x"])
    in_maps = make_core_inputs(**inputs)
    res = run_on_cores(in_maps)
    return assemble(res, b=x.shape[0])
